# revision 1
# baseline (speedup 1.0000x reference)
"""Trainium2 Bass kernel for a single-head cross-attention block.

Reference computation (per batch b of B=128):
    q = input[b] @ Wq            # [T,H]   T=512, C=384, H=64
    k = x[b] @ Wk                # [T,H]
    v = x[b] @ Wv                # [T,H]
    S = (q @ k.T) * C**-0.5      # [T,T], causal mask
    P = softmax(S, axis=-1)
    out[b] = P @ v               # [T,H]

Strategy: data-parallel over 8 NeuronCores (16 batches each). Host-side we
pre-transpose input/x to [C,T] (the PE contracts along partitions, so the
projections need C on partitions) and cast to bf16. On device, per batch:

  - qT/kT = Wq'.T @ inpT / Wk'.T @ xT        -> PSUM [64,512] each
  - v[t]  = xT[:,tchunk].T @ Wv'             -> PSUM [128,64] x4
  - S^T[m] = kT[:,mchunk].T @ qT             -> PSUM [128, 512-128m]
    (S^T layout [k,q]: causal keeps q >= k, so chunk m only needs
     columns 128m..512; the diagonal 128x128 block is masked with a
     precomputed upper-triangular 0/1 tile)
  - E = exp(S^T * scale) on ScalarE (scale fused into the activation)
    No max-subtraction needed: scores are N(0, 0.41^2)-ish, |s|<~3.
  - out_ps[t] += E[m][:,tchunk].T @ [v[m] | 1]   (ones column makes the
    softmax denominator fall out of the same matmuls)
  - out = out_ps[:, :H] * (1/denom)  per-partition scalar, then DMA out.
"""

import numpy as np
import ml_dtypes

import concourse.bass as bass
import concourse.tile as tile
import concourse.mybir as mybir
from concourse.vector_clock import ScopedClock
from concourse.bass_utils import run_bass_kernel_spmd
from concourse.masks import make_upper_triangular

N_CORES = 8
B, T, C, H = 128, 512, 384, 64
BPC = B // N_CORES          # batches per core
CK = C // 128               # contraction chunks for projections
TK = T // 128               # T chunks
SCALE = float(C) ** -0.5
BF16 = mybir.dt.bfloat16
F32 = mybir.dt.float32
EXP = mybir.ActivationFunctionType.Exp

_bf16 = ml_dtypes.bfloat16


def _split_multi_waits(nc: bass.Bass):
    """walrus in this build encodes at most ONE sync-wait per instruction.
    Tile's wait-assignment can attach several. Move the extras onto
    same-engine NOPs inserted immediately before each instruction —
    identical semantics (the engine blocks on the NOP waits first)."""
    n = 0
    for bb in nc.m.functions[0].blocks:
        new_insts = []
        for inst in bb.instructions:
            si = inst.sync_info
            waits = list(si.on_wait) if si and si.on_wait else []
            if len(waits) > 1:
                for w in waits[:-1]:
                    nop = mybir.InstNoOp(name=f"WSPLIT-{n}", ins=[], outs=[])
                    n += 1
                    nop.engine = inst.engine
                    nop.sync_info = mybir.SyncInfo(on_wait=[w], on_update=[])
                    new_insts.append(nop)
                si.on_wait = waits[-1:]
            new_insts.append(inst)
        bb.instructions[:] = new_insts


def build_kernel() -> bass.Bass:
    nc = bass.Bass()
    inpT = nc.dram_tensor("inpT", [BPC, C, T], BF16, kind="ExternalInput")
    xT = nc.dram_tensor("xT", [BPC, C, T], BF16, kind="ExternalInput")
    wq = nc.dram_tensor("wq", [C, H], BF16, kind="ExternalInput")
    wk = nc.dram_tensor("wk", [C, H], BF16, kind="ExternalInput")
    wv = nc.dram_tensor("wv", [C, H], BF16, kind="ExternalInput")
    out = nc.dram_tensor("out", [BPC, T, H], F32, kind="ExternalOutput")

    with tile.TileContext(nc) as tc:
        with (
            tc.tile_pool(name="const", bufs=1) as const_pool,
            tc.tile_pool(name="inputs", bufs=3) as in_pool,
            tc.tile_pool(name="work", bufs=2) as sb_pool,
            tc.tile_pool(name="qk_ps", bufs=1, space="PSUM") as qk_psum,
            tc.tile_pool(name="v_ps", bufs=1, space="PSUM") as v_psum,
            tc.tile_pool(name="st_ps", bufs=3, space="PSUM") as st_psum,
            tc.tile_pool(name="o_ps", bufs=2, space="PSUM") as o_psum,
        ):
            # Constants: weights as [128, CK, H] (C-chunk on partitions), and
            # the upper-triangular (incl. diagonal) 0/1 mask for the causal
            # diagonal blocks of S^T.
            wq_sb = const_pool.tile([128, CK, H], BF16, tag="wq")
            nc.sync.dma_start(wq_sb[:], wq[:, :].rearrange("(c p) h -> p c h", p=128))
            wk_sb = const_pool.tile([128, CK, H], BF16, tag="wk")
            nc.sync.dma_start(wk_sb[:], wk[:, :].rearrange("(c p) h -> p c h", p=128))
            wv_sb = const_pool.tile([128, CK, H], BF16, tag="wv")
            nc.sync.dma_start(wv_sb[:], wv[:, :].rearrange("(c p) h -> p c h", p=128))
            tri = const_pool.tile([128, 128], BF16, tag="tri")
            make_upper_triangular(nc, tri[:], val=1.0, diag=True)

            for b in range(BPC):
                it = in_pool.tile([128, CK, T], BF16, tag="inpT")
                nc.sync.dma_start(it[:], inpT[b].rearrange("(c p) t -> p c t", p=128))
                xt = in_pool.tile([128, CK, T], BF16, tag="xT")
                nc.sync.dma_start(xt[:], xT[b].rearrange("(c p) t -> p c t", p=128))

                # qT | kT in one 2-bank PSUM tile [64, 2T]. Casts are split
                # so the q-cast overlaps the k-projections and the k-cast
                # overlaps the v-matmuls (keeps PE from stalling on S^T).
                qk_ps = qk_psum.tile([H, 2 * T], F32, tag="qk")
                qk_sb = sb_pool.tile([H, 2 * T], BF16, tag="qk_sb")
                for c in range(CK):
                    nc.tensor.matmul(
                        qk_ps[:, 0:T], wq_sb[:, c, :], it[:, c, :],
                        start=(c == 0), stop=(c == CK - 1),
                    )
                nc.vector.tensor_copy(qk_sb[:, 0:T], qk_ps[:, 0:T])
                for c in range(CK):
                    nc.tensor.matmul(
                        qk_ps[:, T:2 * T], wk_sb[:, c, :], xt[:, c, :],
                        start=(c == 0), stop=(c == CK - 1),
                    )
                nc.vector.tensor_copy(qk_sb[:, T:2 * T], qk_ps[:, T:2 * T])

                # v chunks [128, H] x TK in one PSUM bank
                v_ps = v_psum.tile([128, TK, H], F32, tag="v")
                for t in range(TK):
                    for c in range(CK):
                        nc.tensor.matmul(
                            v_ps[:, t, :],
                            xt[:, c, 128 * t:128 * (t + 1)],
                            wv_sb[:, c, :],
                            start=(c == 0), stop=(c == CK - 1),
                        )
                v_sb = sb_pool.tile([128, TK, H + 1], BF16, tag="v_sb")
                nc.vector.tensor_copy(v_sb[:, :, 0:H], v_ps[:])
                nc.gpsimd.memset(v_sb[:, :, H], 1.0)

                qT = qk_sb[:, 0:T]
                kT = qk_sb[:, T:2 * T]

                # S^T chunks -> exp -> (mask diagonal block)
                e_tiles = []
                for m in range(TK):
                    n0 = 128 * m
                    st_ps = st_psum.tile([128, T], F32, tag="st")
                    nc.tensor.matmul(
                        st_ps[:, n0:T],
                        kT[:, n0:n0 + 128],
                        qT[:, n0:T],
                        start=True, stop=True,
                    )
                    e = sb_pool.tile([128, T], BF16, tag=f"e{m}")
                    nc.scalar.activation(e[:, n0:T], st_ps[:, n0:T], EXP, scale=SCALE)
                    nc.vector.tensor_mul(e[:, n0:n0 + 128], e[:, n0:n0 + 128], tri[:])
                    e_tiles.append(e)

                # out accumulation over k-chunks; col H carries the denominator
                o_ps = o_psum.tile([128, TK, H + 1], F32, tag="o")
                for t in range(TK):
                    for m in range(t + 1):
                        nc.tensor.matmul(
                            o_ps[:, t, :],
                            e_tiles[m][:, 128 * t:128 * (t + 1)],
                            v_sb[:, m, :],
                            start=(m == 0), stop=(m == t),
                        )

                # normalize and store
                recip = sb_pool.tile([128, TK], F32, tag="recip")
                nc.vector.reciprocal(recip[:], o_ps[:, :, H])
                o_sb = sb_pool.tile([128, TK, H], F32, tag="o_sb")
                for t in range(TK):
                    nc.scalar.mul(o_sb[:, t, :], o_ps[:, t, 0:H], recip[:, t:t + 1])
                nc.sync.dma_start(
                    out[b].rearrange("(t p) h -> p t h", p=128), o_sb[:]
                )
    _split_multi_waits(nc)
    return nc


_cached_nc = None


def kernel(input: np.ndarray, x: np.ndarray, Wq: np.ndarray, Wk: np.ndarray,
           Wv: np.ndarray) -> np.ndarray:
    global _cached_nc

    input = np.asarray(input, dtype=np.float32)
    x = np.asarray(x, dtype=np.float32)
    inpT = np.transpose(input, (0, 2, 1)).astype(_bf16)   # [B, C, T] bf16
    xT = np.transpose(x, (0, 2, 1)).astype(_bf16)
    wq_b = np.asarray(Wq, dtype=np.float32).astype(_bf16)
    wk_b = np.asarray(Wk, dtype=np.float32).astype(_bf16)
    wv_b = np.asarray(Wv, dtype=np.float32).astype(_bf16)

    if _cached_nc is None:
        _cached_nc = build_kernel()
    nc = _cached_nc

    in_maps = []
    for c in range(N_CORES):
        sl = slice(c * BPC, (c + 1) * BPC)
        in_maps.append({
            "inpT": np.ascontiguousarray(inpT[sl]),
            "xT": np.ascontiguousarray(xT[sl]),
            "wq": wq_b, "wk": wk_b, "wv": wv_b,
        })

    res = run_bass_kernel_spmd(nc, in_maps, core_ids=list(range(N_CORES)))
    out = np.concatenate([r["out"] for r in res.results], axis=0)
    return out.astype(np.float32)



# revision 2
# speedup vs baseline: 1.2691x; 1.2691x over previous
"""Trainium2 Bass kernel for a single-head cross-attention block.

Reference computation (per batch b of B=128):
    q = input[b] @ Wq            # [T,H]   T=512, C=384, H=64
    k = x[b] @ Wk                # [T,H]
    v = x[b] @ Wv                # [T,H]
    S = (q @ k.T) * C**-0.5      # [T,T], causal mask
    P = softmax(S, axis=-1)
    out[b] = P @ v               # [T,H]

Strategy: data-parallel over 8 NeuronCores (16 batches each).

Host-side layouts are partition-major so every DMA moves one large
contiguous run per partition (3 KB/partition per input batch, 1 KB for
the output), instead of hundreds of small scattered packets.

Device, per batch (software-pipelined one stage deep so the PE never
idles waiting on softmax):
  - qT|kT = Wq'.T@inpT / Wk'.T@xT -> one 2-bank PSUM [64, 2T]
  - v[t]  = xT[:,tchunk].T @ Wv'  -> PSUM [128,TK,H]
  - S^T chunks packed into ONE PSUM tile [128, 1280] at bank-aligned
    offsets {m0:0, m1:512, m3:896, m2:1024} (causal: chunk m only needs
    q >= 128m), so exp runs as 3 bank-aligned activations.
  - E = exp(S^T * scale) on ScalarE; diagonal-block causal masks on
    GpSimd (upper-triangular 0/1 tile).
  - PV of the PREVIOUS batch is emitted between this batch's v-matmuls
    and S^T so the exp/mask latency hides under ~2us of PE work.
    A ones-column in v_sb makes the softmax denominator fall out of the
    same matmuls; normalize = DVE reciprocal + per-chunk scalar mul.
"""

import numpy as np
import ml_dtypes

import concourse.bass as bass
import concourse.tile as tile
import concourse.mybir as mybir
from concourse.bass_utils import run_bass_kernel_spmd
from concourse.masks import make_upper_triangular

N_CORES = 8
B, T, C, H = 128, 512, 384, 64
BPC = B // N_CORES          # batches per core
CK = C // 128               # contraction chunks for projections
TK = T // 128               # T chunks
SCALE = float(C) ** -0.5
BF16 = mybir.dt.bfloat16
F32 = mybir.dt.float32
EXP = mybir.ActivationFunctionType.Exp

_bf16 = ml_dtypes.bfloat16

# Packed layout of causal S^T chunks inside one [128, 1280] PSUM tile.
# Chunk m holds S^T[k in 128m:128(m+1), q in 128m:512]  (n = 512-128m).
# Offsets chosen so no matmul output crosses a 2KB PSUM bank boundary:
#   bank0 = m0 (512 f32), bank1 = m1 (384) + m3 (128), bank2 = m2 (256)
ST_BASE = {0: 0, 1: 512, 2: 1024, 3: 896}
ST_N = {0: 512, 1: 384, 2: 256, 3: 128}
ST_PACKED = 1280
# exp over bank-aligned ranges; each covers complete chunks
EXP_RANGES = [(0, 512), (512, 1024), (1024, 1280)]
ST_EMIT_ORDER = [0, 1, 3, 2]


def _split_multi_waits(nc: bass.Bass):
    """walrus in this build encodes at most ONE sync-wait per instruction.
    Tile's wait-assignment can attach several. Move the extras onto
    same-engine NOPs inserted immediately before each instruction —
    identical semantics (the engine blocks on the NOP waits first)."""
    n = 0
    for bb in nc.m.functions[0].blocks:
        new_insts = []
        for inst in bb.instructions:
            si = inst.sync_info
            waits = list(si.on_wait) if si and si.on_wait else []
            if len(waits) > 1:
                for w in waits[:-1]:
                    nop = mybir.InstNoOp(name=f"WSPLIT-{n}", ins=[], outs=[])
                    n += 1
                    nop.engine = inst.engine
                    nop.sync_info = mybir.SyncInfo(on_wait=[w], on_update=[])
                    new_insts.append(nop)
                si.on_wait = waits[-1:]
            new_insts.append(inst)
        bb.instructions[:] = new_insts


def build_kernel() -> bass.Bass:
    nc = bass.Bass()
    inpT = nc.dram_tensor("inpT", [BPC, 128, CK, T], BF16, kind="ExternalInput")
    xT = nc.dram_tensor("xT", [BPC, 128, CK, T], BF16, kind="ExternalInput")
    wall = nc.dram_tensor("wall", [128, 3, CK, H], BF16, kind="ExternalInput")
    out = nc.dram_tensor("out", [BPC, 128, TK, H], F32, kind="ExternalOutput")

    PREFETCH = 3

    with tile.TileContext(nc) as tc:
        with (
            tc.tile_pool(name="const", bufs=1) as const_pool,
            tc.tile_pool(name="inputs", bufs=PREFETCH + 1) as in_pool,
            tc.tile_pool(name="qk", bufs=2) as qk_pool,
            tc.tile_pool(name="e", bufs=2) as e_pool,
            tc.tile_pool(name="vsb", bufs=2) as v_pool,
            tc.tile_pool(name="osb", bufs=4) as o_pool,
            tc.tile_pool(name="misc", bufs=2) as misc_pool,
            tc.tile_pool(name="qk_ps", bufs=1, space="PSUM") as qk_psum,
            tc.tile_pool(name="st_ps", bufs=1, space="PSUM") as st_psum,
            tc.tile_pool(name="v_ps", bufs=1, space="PSUM") as v_psum,
            tc.tile_pool(name="o_ps", bufs=2, space="PSUM") as o_psum,
        ):
            w_sb = const_pool.tile([128, 3, CK, H], BF16, tag="wall")
            nc.sync.dma_start(w_sb[:], wall[:])
            tri = const_pool.tile([128, 128], BF16, tag="tri")
            make_upper_triangular(nc, tri[:], val=1.0, diag=True)

            in_tiles = {}

            def emit_load(b):
                it = in_pool.tile([128, CK, T], BF16, tag="it")
                nc.sync.dma_start(it[:], inpT[b])
                xt = in_pool.tile([128, CK, T], BF16, tag="xt")
                nc.sync.dma_start(xt[:], xT[b])
                in_tiles[b] = (it, xt)

            state = {}

            def emit_qkv(b):
                it, xt = in_tiles.pop(b)
                qk_ps = qk_psum.tile([H, 2 * T], F32, tag="qk")
                qk_sb = qk_pool.tile([H, 2 * T], BF16, tag="qk_sb")
                for c in range(CK):
                    nc.tensor.matmul(
                        qk_ps[:, 0:T], w_sb[:, 0, c, :], it[:, c, :],
                        start=(c == 0), stop=(c == CK - 1),
                    )
                for c in range(CK):
                    nc.tensor.matmul(
                        qk_ps[:, T:2 * T], w_sb[:, 1, c, :], xt[:, c, :],
                        start=(c == 0), stop=(c == CK - 1),
                    )
                nc.vector.tensor_copy(qk_sb[:, 0:T], qk_ps[:, 0:T])
                nc.scalar.copy(qk_sb[:, T:2 * T], qk_ps[:, T:2 * T])

                v_ps = v_psum.tile([128, TK, H], F32, tag="v")
                for t in range(TK):
                    for c in range(CK):
                        nc.tensor.matmul(
                            v_ps[:, t, :],
                            xt[:, c, 128 * t:128 * (t + 1)],
                            w_sb[:, 2, c, :],
                            start=(c == 0), stop=(c == CK - 1),
                        )
                v_sb = v_pool.tile([128, TK, H + 1], BF16, tag="v_sb")
                nc.vector.tensor_copy(v_sb[:, :, 0:H], v_ps[:])
                nc.gpsimd.memset(v_sb[:, :, H], 1.0)
                state[b] = (qk_sb, v_sb)

            def emit_st(b):
                qk_sb, _ = state[b]
                qT = qk_sb[:, 0:T]
                kT = qk_sb[:, T:2 * T]
                st_ps = st_psum.tile([128, ST_PACKED], F32, tag="st")
                for m in ST_EMIT_ORDER:
                    n0 = 128 * m
                    nc.tensor.matmul(
                        st_ps[:, ST_BASE[m]:ST_BASE[m] + ST_N[m]],
                        kT[:, n0:n0 + 128],
                        qT[:, n0:T],
                        start=True, stop=True,
                    )
                e = e_pool.tile([128, ST_PACKED], BF16, tag="e")
                for lo, hi in EXP_RANGES:
                    nc.scalar.activation(e[:, lo:hi], st_ps[:, lo:hi], EXP,
                                         scale=SCALE)
                for m in range(TK):
                    off = ST_BASE[m]
                    nc.gpsimd.tensor_mul(e[:, off:off + 128],
                                         e[:, off:off + 128], tri[:])
                state[b] = (state[b][0], state[b][1], e)

            def emit_pv(b):
                _, v_sb, e = state.pop(b)
                o_ps = o_psum.tile([128, TK, H + 1], F32, tag="o")
                for t in range(TK):
                    for m in range(t + 1):
                        off = ST_BASE[m] + 128 * (t - m)
                        nc.tensor.matmul(
                            o_ps[:, t, :],
                            e[:, off:off + 128],
                            v_sb[:, m, :],
                            start=(m == 0), stop=(m == t),
                        )
                recip = misc_pool.tile([128, TK], F32, tag="recip")
                nc.vector.reciprocal(recip[:], o_ps[:, :, H])
                o_sb = o_pool.tile([128, TK, H], F32, tag="o_sb")
                for t in range(TK):
                    nc.vector.tensor_scalar_mul(
                        o_sb[:, t, :], o_ps[:, t, 0:H], recip[:, t:t + 1])
                nc.sync.dma_start(out[b], o_sb[:])

            for b in range(min(PREFETCH, BPC)):
                emit_load(b)
            for b in range(BPC):
                if b + PREFETCH < BPC:
                    emit_load(b + PREFETCH)
                emit_qkv(b)
                if b > 0:
                    emit_pv(b - 1)
                emit_st(b)
            emit_pv(BPC - 1)
    _split_multi_waits(nc)
    return nc


def _layout_input(a: np.ndarray) -> np.ndarray:
    """[n, T, C] f32 -> [n, 128, CK, T] bf16, partition-major."""
    a = np.asarray(a, dtype=np.float32)
    n = a.shape[0]
    a = a.transpose(0, 2, 1).reshape(n, CK, 128, T).transpose(0, 2, 1, 3)
    return np.ascontiguousarray(a).astype(_bf16)


def _layout_weights(Wq, Wk, Wv) -> np.ndarray:
    """three [C, H] -> [128, 3, CK, H] bf16."""
    def lay(w):
        w = np.asarray(w, dtype=np.float32)
        return w.reshape(CK, 128, H).transpose(1, 0, 2)
    return np.ascontiguousarray(
        np.stack([lay(Wq), lay(Wk), lay(Wv)], axis=1)).astype(_bf16)


def prepare_in_maps(input, x, Wq, Wk, Wv):
    inpT = _layout_input(input)
    xT = _layout_input(x)
    wall = _layout_weights(Wq, Wk, Wv)
    in_maps = []
    for c in range(N_CORES):
        sl = slice(c * BPC, (c + 1) * BPC)
        in_maps.append({
            "inpT": np.ascontiguousarray(inpT[sl]),
            "xT": np.ascontiguousarray(xT[sl]),
            "wall": wall,
        })
    return in_maps


def postprocess(results) -> np.ndarray:
    # device layout [BPC, 128, TK, H]: element [b, p, t, h] = out[b, 128t+p, h]
    outs = [r["out"].reshape(BPC, 128, TK, H).transpose(0, 2, 1, 3)
            .reshape(BPC, T, H) for r in results]
    return np.concatenate(outs, axis=0).astype(np.float32)


_cached_nc = None


def kernel(input: np.ndarray, x: np.ndarray, Wq: np.ndarray, Wk: np.ndarray,
           Wv: np.ndarray) -> np.ndarray:
    global _cached_nc
    if _cached_nc is None:
        _cached_nc = build_kernel()
    nc = _cached_nc
    in_maps = prepare_in_maps(input, x, Wq, Wk, Wv)
    res = run_bass_kernel_spmd(nc, in_maps, core_ids=list(range(N_CORES)))
    return postprocess(res.results)
